# revision 1
# baseline (speedup 1.0000x reference)
"""Self-contained Trainium2 Bass kernel for nn_BuildSubGraph_32615981645853
(MAGNA graph-attention + per-user batch-norm pooling), SPMD over 8 NeuronCores.

Sharding: graph nodes (3000, padded to 3072) are sharded 8-way (384 rows/core).
Each core computes its rows of the masked-softmax attention matrix E and the
PPR-style diffusion hops; the full diffusion state z is re-assembled per hop
with an AllGather (7 total).  The per-user gather/batch-norm/pooling stage
collapses algebraically to
    out[b,g,h] = gamma[h]*inv[b,h]*(S[g,h] - W*mean[b,h]) + beta[h]*W + bp
where
    S[g,h]    = sum_n softmax_g(graph@Wg)[n,g] * Wp[n] * graph[n,h]
    mean[b,h] = (1/200) * sum_{l: cate[b,l]!=0} graph[cate[b,l],h]
    E2[b,h]   = (1/200) * sum_{l: cate[b,l]!=0} s2[cate[b,l]] * graph[.,h]^2
    var       = E2 - mean^2,  inv = 1/sqrt(var+eps),  W = sum_n Wp[n]
(using sum_g softmax = 1 and sum_g softmax^2 = s2).  The user gather becomes a
host-built count matrix so mean/E2 are tiny matmuls contracted over nodes,
sharded over nodes and finished with one AllReduce.
"""

import numpy as np
import ml_dtypes

import concourse.bacc as bacc
import concourse.mybir as mybir
import concourse.tile as tile
from concourse.bass import ts
from concourse.masks import make_identity
from concourse.bass_utils import run_bass_kernel_spmd

BF16 = ml_dtypes.bfloat16

NC = 8
N = 3072
C = N // 128  # 24
OWN = N // NC  # 384
OT = OWN // 128  # 3
H = 64
B = 256
G = 4
L = 50
NUM_CATES = 3000
ALPHA = 0.15
EPS = 1e-5
SCALE = 0.125

F32 = mybir.dt.float32
BF = mybir.dt.bfloat16
RG = [list(range(NC))]


def _build_nc(W_scalar: float):
    nc = bacc.Bacc("TRN2", target_bir_lowering=False, debug=False, num_devices=NC)

    i_hT0 = nc.dram_tensor("hT0_bf", [H, N], BF, kind="ExternalInput")
    i_hT0o = nc.dram_tensor("hT0_own", [H, OWN], BF, kind="ExternalInput")
    i_embo = nc.dram_tensor("emb_own", [OWN, H], F32, kind="ExternalInput")
    i_adjT = nc.dram_tensor("adjT_own", [N, OWN], BF, kind="ExternalInput")
    i_W = [
        nc.dram_tensor(nm, [H, H], BF, kind="ExternalInput")
        for nm in ("Wq1", "Wk1", "Wv1", "Wq2", "Wk2", "Wv2")
    ]
    i_Wg = nc.dram_tensor("Wg_ext", [H + 1, G], F32, kind="ExternalInput")
    i_Wpo = nc.dram_tensor("Wp_own", [OWN, 1], F32, kind="ExternalInput")
    i_cnt = nc.dram_tensor("countsT_own", [OWN, B], F32, kind="ExternalInput")
    i_gam = nc.dram_tensor("gamma_row", [1, H], F32, kind="ExternalInput")
    i_bet = nc.dram_tensor("beta_eff_row", [1, H], F32, kind="ExternalInput")
    o_out = nc.dram_tensor("out", [B, G, H], F32, kind="ExternalOutput")

    with tile.TileContext(nc) as tc:
        with (
            tc.tile_pool(name="const", bufs=1) as constp,
            tc.tile_pool(name="big", bufs=1) as bigp,
            tc.tile_pool(name="lay", bufs=2) as layp,
            tc.tile_pool(name="own", bufs=2) as ownp,
            tc.tile_pool(name="zpool", bufs=2) as zp,
            tc.tile_pool(name="sm", bufs=2) as smp,
            tc.tile_pool(name="psA", bufs=3, space="PSUM") as pstp,
            tc.tile_pool(name="psB", bufs=2, space="PSUM") as phopp,
            tc.tile_pool(name="psC", bufs=1, space="PSUM") as pqp,
            tc.tile_pool(name="psD", bufs=2, space="PSUM") as pmp,
            tc.tile_pool(name="dram", bufs=1, space="DRAM") as dramp,
        ):
            idf = constp.tile([128, 128], F32)
            make_identity(nc, idf[:])
            idb = constp.tile([128, 128], BF)
            nc.vector.tensor_copy(idb[:], idf[:])
            epsc = constp.tile([128, 1], F32)
            nc.vector.memset(epsc[:], EPS)

            # ---- persistent loads ----
            hT0 = constp.tile([H, N], BF)
            nc.sync.dma_start(hT0[:], i_hT0.ap())
            hT0o = constp.tile([H, OWN], BF)
            nc.sync.dma_start(hT0o[:], i_hT0o.ap())
            Wsb = []
            for t in i_W:
                w = constp.tile([H, H], BF, name=f"w_{t.name}")
                nc.sync.dma_start(w[:], t.ap())
                Wsb.append(w)
            adjT = bigp.tile([128, C, OWN], BF)
            for qq in range(4):
                nc.sync.dma_start(
                    adjT[:, qq * 6 : (qq + 1) * 6, :],
                    i_adjT.ap().rearrange("(c p) i -> p c i", p=128)[
                        :, qq * 6 : (qq + 1) * 6, :
                    ],
                )
            gam = constp.tile([128, H], F32)
            nc.sync.dma_start(gam[:], i_gam.ap()[0:1, :].partition_broadcast(128))
            bet = constp.tile([128, H], F32)
            nc.sync.dma_start(bet[:], i_bet.ap()[0:1, :].partition_broadcast(128))
            h0o = ownp.tile([128, OT, H], F32, tag="resid")
            nc.sync.dma_start(h0o[:], i_embo.ap().rearrange("(t p) f -> p t f", p=128))

            E = bigp.tile([128, C, OWN], BF)

            hT, hTo, res = hT0, hT0o, h0o
            hfinal = h0o
            for lay in range(2):
                Wq, Wk, Wv = Wsb[3 * lay : 3 * lay + 3]
                # qT_own [H, OWN]
                pq0 = pqp.tile([H, 512], F32, tag="pq", name=f"pq0_{lay}")
                nc.tensor.matmul(pq0[:, :OWN], Wq[:], hTo[:], start=True, stop=True)
                qTo = layp.tile([H, OWN], BF, tag="qto")
                nc.scalar.copy(qTo[:], pq0[:, :OWN])
                # kT full [H, N]
                kT = layp.tile([H, N], BF, tag="kt")
                for bb in range(N // 512):
                    pk = pqp.tile([H, 512], F32, tag="pq", name=f"pk{lay}_{bb}")
                    nc.tensor.matmul(
                        pk[:], Wk[:], hT[:, ts(bb, 512)], start=True, stop=True
                    )
                    nc.vector.tensor_copy(kT[:, ts(bb, 512)], pk[:])
                # v full (natural) + ones column (produces row-sum d in hop 0)
                vsb = layp.tile([128, C, H + 1], BF, tag="v")
                nc.vector.memset(vsb[:, :, H], 1.0)
                for jc in range(C):
                    pv = pmp.tile([128, H], F32, tag="pm", name=f"pv{lay}_{jc}")
                    nc.tensor.matmul(
                        pv[:], hT[:, ts(jc, 128)], Wv[:], start=True, stop=True
                    )
                    nc.vector.tensor_copy(vsb[:, jc, :H], pv[:])
                # alpha*v for own rows (f32)
                avo = ownp.tile([128, OT, H], F32, tag="avo")
                for it in range(OT):
                    pv = pmp.tile([128, H], F32, tag="pm", name=f"pvo{lay}_{it}")
                    nc.tensor.matmul(
                        pv[:], hTo[:, ts(it, 128)], Wv[:], start=True, stop=True
                    )
                    nc.scalar.mul(avo[:, it, :], pv[:], ALPHA)

                # scores^T (own cols), exp, mask -> E
                for jc in range(C):
                    pst = pstp.tile([128, OWN], F32, tag="pst", name=f"pst{lay}_{jc}")
                    nc.tensor.matmul(
                        pst[:], kT[:, ts(jc, 128)], qTo[:], start=True, stop=True
                    )
                    nc.scalar.activation(
                        E[:, jc, :],
                        pst[:],
                        mybir.ActivationFunctionType.Exp,
                        scale=SCALE,
                    )
                    nc.vector.tensor_mul(E[:, jc, :], E[:, jc, :], adjT[:, jc, :])

                # diffusion hops
                wsc = ownp.tile([128, OT], F32, tag="wsc")
                zprev = None
                for hop in range(4):
                    width = H + 1 if hop == 0 else H
                    znew = ownp.tile(
                        [128, OT, H], F32, tag="znew", name=f"zn{lay}{hop}"
                    )
                    zbfo = ownp.tile(
                        [128, OT, H], BF, tag="zbfo", name=f"zb{lay}{hop}"
                    )
                    for it in range(OT):
                        ph = phopp.tile(
                            [128, H + 1], F32, tag="ph", name=f"ph{lay}{hop}{it}"
                        )
                        for jc in range(C):
                            rhs = vsb[:, jc, :] if hop == 0 else zprev[:, jc, :]
                            nc.tensor.matmul(
                                ph[:, :width],
                                E[:, jc, ts(it, 128)],
                                rhs,
                                start=(jc == 0),
                                stop=(jc == C - 1),
                            )
                        if hop == 0:
                            nc.vector.tensor_scalar_mul(
                                wsc[:, it : it + 1],
                                ph[:, H : H + 1],
                                1.0 / (1.0 - ALPHA),
                            )
                            nc.vector.reciprocal(
                                wsc[:, it : it + 1], wsc[:, it : it + 1]
                            )
                        nc.vector.tensor_scalar(
                            znew[:, it, :],
                            ph[:, :H],
                            wsc[:, it : it + 1],
                            None,
                            mybir.AluOpType.mult,
                        )
                        nc.vector.tensor_add(
                            znew[:, it, :], znew[:, it, :], avo[:, it, :]
                        )
                        nc.vector.tensor_copy(zbfo[:, it, :], znew[:, it, :])

                    last_hop = hop == 3
                    if last_hop:
                        hres = ownp.tile(
                            [128, OT, H], F32, tag="resid", name=f"hres{lay}"
                        )
                        hfinal = hres
                        for it in range(OT):
                            nc.vector.tensor_add(
                                hres[:, it, :], res[:, it, :], znew[:, it, :]
                            )

                    if not (lay == 1 and last_hop):
                        ccin = dramp.tile([OWN, H], BF, tag="ccin", bufs=2)
                        ccout = dramp.tile(
                            [N, H], BF, addr_space="Shared", tag="ccout", bufs=2
                        )
                        nc.sync.dma_start(
                            ccin[:].rearrange("(t p) f -> p t f", p=128), zbfo[:]
                        )
                        nc.gpsimd.collective_compute(
                            "AllGather",
                            mybir.AluOpType.bypass,
                            replica_groups=RG,
                            ins=[ccin.opt()],
                            outs=[ccout.opt()],
                        )
                        if not last_hop:
                            zn = zp.tile([128, C, H], BF, tag="z")
                            nc.sync.dma_start(
                                zn[:], ccout[:].rearrange("(c p) f -> p c f", p=128)
                            )
                            zprev = zn
                        else:
                            # layer boundary: hT1 = hT0 + z4^T
                            zn4 = zp.tile([128, C, H], BF, tag="z")
                            for qq in range(4):
                                nc.sync.dma_start(
                                    zn4[:, qq * 6 : (qq + 1) * 6, :],
                                    ccout[:].rearrange("(c p) f -> p c f", p=128)[
                                        :, qq * 6 : (qq + 1) * 6, :
                                    ],
                                )
                            zT = layp.tile([H, N], BF, tag="zT")
                            for jc in range(C):
                                ptz = pmp.tile(
                                    [H, 128], BF, tag="pm", name=f"ptz{lay}_{jc}"
                                )
                                nc.tensor.transpose(ptz[:], zn4[:, jc, :], idb[:])
                                nc.vector.tensor_copy(zT[:, ts(jc, 128)], ptz[:])
                            hT1 = layp.tile([H, N], BF, tag="ht1")
                            nc.vector.tensor_add(hT1[:], hT0[:], zT[:])
                            zTo = layp.tile([H, OWN], BF, tag="zto")
                            for it in range(OT):
                                ptq = pmp.tile(
                                    [H, 128], BF, tag="pm", name=f"ptq{lay}{it}"
                                )
                                nc.tensor.transpose(ptq[:], zbfo[:, it, :], idb[:])
                                nc.vector.tensor_copy(zTo[:, ts(it, 128)], ptq[:])
                            hT1o = layp.tile([H, OWN], BF, tag="ht1o")
                            nc.vector.tensor_add(hT1o[:], hT0o[:], zTo[:])
                            hT, hTo, res = hT1, hT1o, hres

            # ================= end stage =================
            graph = hfinal
            gTe = layp.tile([H + 1, OWN], F32, tag="gte")
            nc.vector.memset(gTe[H : H + 1, :], 1.0)
            for it in range(OT):
                pt = pmp.tile([H, 128], F32, tag="pm", name=f"gt{it}")
                nc.tensor.transpose(pt[:], graph[:, it, :], idf[:])
                nc.vector.tensor_copy(gTe[:H, ts(it, 128)], pt[:])
            Wgsb = constp.tile([H + 1, G], F32)
            nc.sync.dma_start(Wgsb[:], i_Wg.ap())
            wpsb = constp.tile([128, OT], F32)
            nc.sync.dma_start(
                wpsb[:][:, :, None],
                i_Wpo.ap().rearrange("(t p) f -> p t f", p=128),
            )
            cnt = bigp.tile([128, OT, B], F32)
            nc.sync.dma_start(
                cnt[:], i_cnt.ap().rearrange("(t p) b -> p t b", p=128)
            )

            sc = smp.tile([128, OT, G], F32, tag="sc", bufs=1)
            s2 = smp.tile([128, OT], F32, tag="s2", bufs=1)
            wsc2 = smp.tile([128, OT, G], F32, tag="wsc2", bufs=1)
            g2 = smp.tile([128, OT, H], F32, tag="g2", bufs=1)
            c2 = bigp.tile([128, OT, B], F32)
            for it in range(OT):
                pc = pmp.tile([128, G], F32, tag="pm", name=f"pc{it}")
                nc.tensor.matmul(
                    pc[:], gTe[:, ts(it, 128)], Wgsb[:], start=True, stop=True
                )
                nc.scalar.activation(
                    sc[:, it, :], pc[:], mybir.ActivationFunctionType.Exp
                )
                rs = smp.tile([128, 1], F32, tag="rs")
                nc.vector.tensor_reduce(
                    rs[:],
                    sc[:, it, :],
                    axis=mybir.AxisListType.X,
                    op=mybir.AluOpType.add,
                )
                nc.vector.reciprocal(rs[:], rs[:])
                nc.vector.tensor_scalar(
                    sc[:, it, :], sc[:, it, :], rs[:], None, mybir.AluOpType.mult
                )
                sq = smp.tile([128, G], F32, tag="sq")
                nc.vector.tensor_mul(sq[:], sc[:, it, :], sc[:, it, :])
                nc.vector.tensor_reduce(
                    s2[:, it : it + 1],
                    sq[:],
                    axis=mybir.AxisListType.X,
                    op=mybir.AluOpType.add,
                )
                nc.vector.tensor_scalar(
                    wsc2[:, it, :],
                    sc[:, it, :],
                    wpsb[:, it : it + 1],
                    None,
                    mybir.AluOpType.mult,
                )
                nc.vector.tensor_mul(g2[:, it, :], graph[:, it, :], graph[:, it, :])
                nc.vector.tensor_scalar(
                    c2[:, it, :],
                    cnt[:, it, :],
                    s2[:, it : it + 1],
                    None,
                    mybir.AluOpType.mult,
                )

            packsb = smp.tile([128, 5, H], F32, tag="pack", bufs=1)
            pS = pmp.tile([G, H], F32, tag="pm", name="pS")
            for it in range(OT):
                nc.tensor.matmul(
                    pS[:],
                    wsc2[:, it, :],
                    graph[:, it, :],
                    start=(it == 0),
                    stop=(it == OT - 1),
                )
            nc.vector.memset(packsb[:, 4, :], 0.0)
            nc.vector.tensor_copy(packsb[:G, 4, :], pS[:])
            for bt in range(2):
                pmean = pmp.tile([128, H], F32, tag="pm", name=f"pmean{bt}")
                for it in range(OT):
                    nc.tensor.matmul(
                        pmean[:],
                        cnt[:, it, ts(bt, 128)],
                        graph[:, it, :],
                        start=(it == 0),
                        stop=(it == OT - 1),
                    )
                nc.scalar.copy(packsb[:, bt, :], pmean[:])
            for bt in range(2):
                pE2 = pmp.tile([128, H], F32, tag="pm", name=f"pE2{bt}")
                for it in range(OT):
                    nc.tensor.matmul(
                        pE2[:],
                        c2[:, it, ts(bt, 128)],
                        g2[:, it, :],
                        start=(it == 0),
                        stop=(it == OT - 1),
                    )
                nc.scalar.copy(packsb[:, 2 + bt, :], pE2[:])

            arin = dramp.tile([2 * B + G, H], F32)
            arout = dramp.tile([2 * B + G, H], F32, addr_space="Shared")
            nc.sync.dma_start(
                arin[: 2 * B, :].rearrange("(x p) f -> p x f", p=128),
                packsb[:, :4, :],
            )
            nc.sync.dma_start(arin[2 * B :, :], packsb[:G, 4, :])
            nc.gpsimd.collective_compute(
                "AllReduce",
                mybir.AluOpType.add,
                replica_groups=RG,
                ins=[arin.opt()],
                outs=[arout.opt()],
            )

            # finalize (identical on every core; host reads core 0)
            Ssb = smp.tile([128, G, H], F32, tag="Ssb", bufs=1)
            for g in range(G):
                nc.sync.dma_start(
                    Ssb[:, g, :],
                    arout[2 * B + g : 2 * B + g + 1, :].partition_broadcast(128),
                )
            meanf = smp.tile([128, 2, H], F32, tag="meanf", bufs=1)
            e2f = smp.tile([128, 2, H], F32, tag="e2f", bufs=1)
            for bt in range(2):
                nc.sync.dma_start(meanf[:, bt, :], arout[ts(bt, 128), :])
                nc.sync.dma_start(
                    e2f[:, bt, :], arout[B + bt * 128 : B + (bt + 1) * 128, :]
                )
            var = smp.tile([128, 2, H], F32, tag="var", bufs=1)
            nc.vector.tensor_mul(var[:], meanf[:], meanf[:])
            nc.vector.tensor_sub(var[:], e2f[:], var[:])
            sd = smp.tile([128, 2, H], F32, tag="sd", bufs=1)
            nc.scalar.activation(
                sd[:].rearrange("p a b -> p (a b)"),
                var[:].rearrange("p a b -> p (a b)"),
                mybir.ActivationFunctionType.Sqrt,
                bias=epsc[:],
            )
            Am = smp.tile([128, 2, H], F32, tag="Am", bufs=1)
            nc.vector.reciprocal(Am[:], sd[:])
            for bt in range(2):
                nc.vector.tensor_mul(Am[:, bt, :], Am[:, bt, :], gam[:])
            Bc = smp.tile([128, 2, H], F32, tag="Bc", bufs=1)
            nc.vector.tensor_mul(Bc[:], Am[:], meanf[:])
            nc.vector.tensor_scalar_mul(Bc[:], Bc[:], -W_scalar)
            for bt in range(2):
                nc.vector.tensor_add(Bc[:, bt, :], Bc[:, bt, :], bet[:])
            for bt in range(2):
                og = smp.tile([128, G, H], F32, tag="og")
                nc.vector.tensor_mul(
                    og[:], Ssb[:], Am[:, bt, None, :].broadcast_to([128, G, H])
                )
                nc.vector.tensor_add(
                    og[:], og[:], Bc[:, bt, None, :].broadcast_to([128, G, H])
                )
                nc.sync.dma_start(o_out.ap()[ts(bt, 128), :, :], og[:])

    nc.compile()
    return nc


def _prep_inputs(inputs):
    cate = np.asarray(inputs["cate_list"])
    adj = np.asarray(inputs["adj"], np.float32)
    emb = np.asarray(inputs["emb"], np.float32)
    Wq = np.asarray(inputs["Wq"], np.float32)
    Wk = np.asarray(inputs["Wk"], np.float32)
    Wv = np.asarray(inputs["Wv"], np.float32)
    Wg = np.asarray(inputs["Wg"], np.float32)
    bg = np.asarray(inputs["bg"], np.float32)
    Wp = np.asarray(inputs["Wp"], np.float32)
    bp = np.asarray(inputs["bp"], np.float32)
    gamma = np.asarray(inputs["gamma"], np.float32)
    beta = np.asarray(inputs["beta"], np.float32)

    adjP = np.zeros((N, N), np.float32)
    adjP[:NUM_CATES, :NUM_CATES] = adj
    idx = np.arange(NUM_CATES, N)
    adjP[idx, idx] = 1.0

    embP = np.zeros((N, H), np.float32)
    embP[:NUM_CATES] = emb
    hT0 = np.ascontiguousarray(embP.T).astype(BF16)

    WpP = np.zeros((N, 1), np.float32)
    WpP[:NUM_CATES] = Wp
    W = float(Wp.sum())
    beta_eff = (beta * W + bp).astype(np.float32).reshape(1, H)
    Wg_ext = np.concatenate([Wg, bg.reshape(1, G)], axis=0).astype(np.float32)

    counts = np.zeros((B, N), np.float32)
    bi = np.repeat(np.arange(B), L)
    ci = cate.reshape(-1).astype(np.int64)
    msk = (ci != 0).astype(np.float32) / float(G * L)
    np.add.at(counts, (bi, ci), msk)
    countsT = np.ascontiguousarray(counts.T)

    in_maps = []
    for c in range(NC):
        sl = slice(c * OWN, (c + 1) * OWN)
        in_maps.append(
            {
                "hT0_bf": hT0,
                "hT0_own": np.ascontiguousarray(hT0[:, sl]),
                "emb_own": np.ascontiguousarray(embP[sl]),
                "adjT_own": np.ascontiguousarray(adjP[sl].T).astype(BF16),
                "Wq1": Wq[0].astype(BF16),
                "Wk1": Wk[0].astype(BF16),
                "Wv1": Wv[0].astype(BF16),
                "Wq2": Wq[1].astype(BF16),
                "Wk2": Wk[1].astype(BF16),
                "Wv2": Wv[1].astype(BF16),
                "Wg_ext": Wg_ext,
                "Wp_own": np.ascontiguousarray(WpP[sl]),
                "countsT_own": np.ascontiguousarray(countsT[sl]),
                "gamma_row": gamma.reshape(1, H).astype(np.float32),
                "beta_eff_row": beta_eff,
            }
        )
    return in_maps, W


_NC_CACHE = {}


def kernel(**inputs) -> np.ndarray:
    in_maps, W = _prep_inputs(inputs)
    key = round(W, 10)
    nc = _NC_CACHE.get(key)
    if nc is None:
        nc = _build_nc(W)
        _NC_CACHE[key] = nc
    res = run_bass_kernel_spmd(nc, in_maps, core_ids=list(range(NC)))
    return np.asarray(res.results[0]["out"], np.float32)



# revision 14
# speedup vs baseline: 1.0874x; 1.0874x over previous
"""Self-contained Trainium2 Bass kernel for nn_BuildSubGraph_32615981645853
(MAGNA graph-attention + per-user batch-norm pooling), SPMD over 8 NeuronCores.

Sharding: graph nodes (3000, padded to 3072) are sharded 8-way (384 rows/core).
Each core computes its rows of the masked-softmax attention matrix E and the
PPR-style diffusion hops; the full diffusion state z is re-assembled per hop
with an AllGather.  Layer-0 q/k/v are precomputed on the host; the layer
boundary AllGather carries layer-1 k^T and v (computed per-core on own rows)
instead of z4, so the full h1 never needs to be assembled.  The per-user
gather/batch-norm/pooling stage collapses algebraically to
    out[b,g,h] = gamma[h]*inv[b,h]*(S[g,h] - W*mean[b,h]) + beta[h]*W + bp
where
    S[g,h]    = sum_n softmax_g(graph@Wg)[n,g] * Wp[n] * graph[n,h]
    mean[b,h] = (1/200) * sum_{l: cate[b,l]!=0} graph[cate[b,l],h]
    E2[b,h]   = (1/200) * sum_{l: cate[b,l]!=0} s2[cate[b,l]] * graph[.,h]^2
    var       = E2 - mean^2,  inv = 1/sqrt(var+eps),  W = sum_n Wp[n]
(using sum_g softmax = 1 and sum_g softmax^2 = s2).  The user gather becomes a
host-built count matrix so mean/E2 are tiny matmuls contracted over nodes,
sharded over nodes and finished with one AllReduce.
"""

import numpy as np
import ml_dtypes

import concourse.bacc as bacc
import concourse.mybir as mybir
import concourse.tile as tile
from concourse.bass import ts
from concourse.masks import make_identity
from concourse.bass_utils import run_bass_kernel_spmd

BF16 = ml_dtypes.bfloat16

NC = 8
N = 3072
C = N // 128  # 24
OWN = N // NC  # 384
OT = OWN // 128  # 3
H = 64
B = 256
G = 4
L = 50
NUM_CATES = 3000
ALPHA = 0.15
EPS = 1e-5
SCALE = 0.125
KVW = H * OWN  # 24576, flat words per kv-cc row

F32 = mybir.dt.float32
BF = mybir.dt.bfloat16
RG = [list(range(NC))]


def _build_nc(W_scalar: float):
    nc = bacc.Bacc("TRN2", target_bir_lowering=False, debug=False, num_devices=NC)

    i_k0T = nc.dram_tensor("k0T_bf", [H, N], BF, kind="ExternalInput")
    i_q0To = nc.dram_tensor("q0To_bf", [H, OWN], BF, kind="ExternalInput")
    i_v0 = nc.dram_tensor("v0_bf", [N, H], BF, kind="ExternalInput")
    i_avo0 = nc.dram_tensor("avo0", [OWN, H], F32, kind="ExternalInput")
    i_embo = nc.dram_tensor("emb_own", [OWN, H], F32, kind="ExternalInput")
    i_adjT = nc.dram_tensor("adjT_own", [N, OWN], BF, kind="ExternalInput")
    i_W2 = [
        nc.dram_tensor(nm, [H, H], BF, kind="ExternalInput")
        for nm in ("Wq2", "Wk2", "Wv2")
    ]
    i_Wg = nc.dram_tensor("Wg_ext", [H + 1, G], F32, kind="ExternalInput")
    i_Wpo = nc.dram_tensor("Wp_own", [OWN, 1], F32, kind="ExternalInput")
    i_cnt = nc.dram_tensor("countsT_own", [OWN, B], F32, kind="ExternalInput")
    i_gam = nc.dram_tensor("gamma_row", [1, H], F32, kind="ExternalInput")
    i_bet = nc.dram_tensor("beta_eff_row", [1, H], F32, kind="ExternalInput")
    o_out = nc.dram_tensor("out", [B, G, H], F32, kind="ExternalOutput")

    with tile.TileContext(nc) as tc:
        with (
            tc.tile_pool(name="const", bufs=1) as constp,
            tc.tile_pool(name="big", bufs=1) as bigp,
            tc.tile_pool(name="lay", bufs=2) as layp,
            tc.tile_pool(name="own", bufs=2) as ownp,
            tc.tile_pool(name="zpool", bufs=2) as zp,
            tc.tile_pool(name="sm", bufs=2) as smp,
            tc.tile_pool(name="psA", bufs=3, space="PSUM") as pstp,
            tc.tile_pool(name="psB", bufs=2, space="PSUM") as phopp,
            tc.tile_pool(name="psC", bufs=1, space="PSUM") as pqp,
            tc.tile_pool(name="psD", bufs=2, space="PSUM") as pmp,
            tc.tile_pool(name="dram", bufs=1, space="DRAM") as dramp,
        ):
            def _sin(nm):
                return nm, nc.enter_named_scope(nm, False)[0]

            def _sout(tok):
                nc.leave_named_scope(tok[0], tok[1], False)

            _t = _sin("load")
            # ---- priority loads: layer-0 k/q first (E gate), adj chunks ----
            k0T = constp.tile([H, N], BF)
            nc.sync.dma_start(k0T[:], i_k0T.ap())
            q0To = constp.tile([H, OWN], BF)
            nc.sync.dma_start(q0To[:], i_q0To.ap())
            adjT = bigp.tile([128, C, OWN], BF)
            for qq in range(4):
                nc.sync.dma_start(
                    adjT[:, qq * 6 : (qq + 1) * 6, :],
                    i_adjT.ap().rearrange("(c p) i -> p c i", p=128)[
                        :, qq * 6 : (qq + 1) * 6, :
                    ],
                )
            # v0 straight into the hop-0 rhs layout (ones column appended)
            vsb0 = layp.tile([128, C, H + 1], BF, tag="v", name="vsb0")
            nc.sync.dma_start(
                vsb0[:, :, :H], i_v0.ap().rearrange("(c p) f -> p c f", p=128)
            )
            nc.vector.memset(vsb0[:, :, H], 1.0)
            avo0 = ownp.tile([128, OT, H], F32, tag="avo", name="avo0")
            nc.sync.dma_start(avo0[:], i_avo0.ap().rearrange("(t p) f -> p t f", p=128))
            h0o = ownp.tile([128, OT, H], F32, tag="resid")
            nc.sync.dma_start(h0o[:], i_embo.ap().rearrange("(t p) f -> p t f", p=128))
            W2sb = []
            for t in i_W2:
                w = constp.tile([H, H], BF, name=f"w_{t.name}")
                nc.sync.dma_start(w[:], t.ap())
                W2sb.append(w)

            idf = constp.tile([128, 128], F32)
            make_identity(nc, idf[:])
            idb = constp.tile([128, 128], BF)
            nc.vector.tensor_copy(idb[:], idf[:])
            epsc = constp.tile([128, 1], F32)
            nc.vector.memset(epsc[:], EPS)
            gam = constp.tile([128, H], F32)
            nc.sync.dma_start(gam[:], i_gam.ap()[0:1, :].partition_broadcast(128))
            bet = constp.tile([128, H], F32)
            nc.sync.dma_start(bet[:], i_bet.ap()[0:1, :].partition_broadcast(128))
            Wgsb = constp.tile([H + 1, G], F32)
            nc.sync.dma_start(Wgsb[:], i_Wg.ap())
            wpsb = constp.tile([128, OT], F32)
            nc.sync.dma_start(
                wpsb[:][:, :, None],
                i_Wpo.ap().rearrange("(t p) f -> p t f", p=128),
            )
            cnt = bigp.tile([128, OT, B], F32)
            nc.sync.dma_start(cnt[:], i_cnt.ap().rearrange("(t p) b -> p t b", p=128))

            E = bigp.tile([128, C, OWN], BF)
            _sout(_t)

            res = h0o
            hfinal = h0o
            kT8 = None
            qTo, vsb, avo = q0To, vsb0, avo0
            for lay in range(2):
                # ---- E = exp(scores/8) * adj  (transposed: [node, own]) ----
                _t = _sin(f"L{lay}.E")
                for jc in range(C):
                    if lay == 0:
                        kchunk = k0T[:, ts(jc, 128)]
                    else:
                        kchunk = kT8[:, jc // OT, (jc % OT) * 128 : (jc % OT + 1) * 128]
                    pst = pstp.tile([128, OWN], F32, tag="pst", name=f"pst{lay}_{jc}")
                    nc.tensor.matmul(pst[:], kchunk, qTo[:], start=True, stop=True)
                    nc.scalar.activation(
                        E[:, jc, :],
                        pst[:],
                        mybir.ActivationFunctionType.Exp,
                        scale=SCALE,
                    )
                    nc.vector.tensor_mul(E[:, jc, :], E[:, jc, :], adjT[:, jc, :])
                _sout(_t)

                # ---- diffusion hops ----
                wsc = ownp.tile([128, OT], F32, tag="wsc", name=f"wsc{lay}")
                winv = ownp.tile([128, OT], F32, tag="winv", name=f"winv{lay}")
                avs = ownp.tile([128, OT, H], BF, tag="avs", name=f"avs{lay}")
                zprev = None
                for hop in range(4):
                    _t = _sin(f"L{lay}.h{hop}")
                    last_hop = hop == 3
                    zbfo = None
                    if not last_hop:
                        zbfo = ownp.tile(
                            [128, OT, H], BF, tag="zbfo", name=f"zb{lay}{hop}"
                        )
                    if last_hop:
                        hres = ownp.tile(
                            [128, OT, H], F32, tag="resid", name=f"hres{lay}"
                        )
                        hfinal = hres
                    for it in range(OT):
                        ph = phopp.tile(
                            [128, H + 1], F32, tag="ph", name=f"ph{lay}{hop}{it}"
                        )
                        if hop == 0:
                            for jc in range(C):
                                nc.tensor.matmul(
                                    ph[:, : H + 1],
                                    E[:, jc, ts(it, 128)],
                                    vsb[:, jc, :],
                                    start=(jc == 0),
                                    stop=(jc == C - 1),
                                )
                            # winv = d/(1-a); wsc = (1-a)/d
                            nc.vector.tensor_scalar_mul(
                                winv[:, it : it + 1],
                                ph[:, H : H + 1],
                                1.0 / (1.0 - ALPHA),
                            )
                            nc.vector.reciprocal(
                                wsc[:, it : it + 1], winv[:, it : it + 1]
                            )
                            zt0 = smp.tile([128, H], F32, tag="zt0")
                            nc.vector.tensor_scalar(
                                zt0[:],
                                ph[:, :H],
                                wsc[:, it : it + 1],
                                None,
                                mybir.AluOpType.mult,
                            )
                            nc.vector.tensor_add(
                                zbfo[:, it, :], zt0[:], avo[:, it, :]
                            )
                            # avs = alpha*v/wsc (PSUM preload for hops 1-3)
                            nc.vector.tensor_scalar(
                                avs[:, it, :],
                                avo[:, it, :],
                                winv[:, it : it + 1],
                                None,
                                mybir.AluOpType.mult,
                            )
                        else:
                            nc.tensor.matmul(
                                ph[:, :H],
                                idb[:],
                                avs[:, it, :],
                                start=True,
                                stop=False,
                            )
                            for jc in range(C):
                                nc.tensor.matmul(
                                    ph[:, :H],
                                    E[:, jc, ts(it, 128)],
                                    zprev[:, jc, :],
                                    start=False,
                                    stop=(jc == C - 1),
                                )
                            if not last_hop:
                                nc.scalar.activation(
                                    zbfo[:, it, :],
                                    ph[:, :H],
                                    mybir.ActivationFunctionType.Copy,
                                    scale=wsc[:, it : it + 1],
                                )
                            else:
                                t1 = smp.tile([128, H], F32, tag="zt0")
                                nc.vector.tensor_scalar(
                                    t1[:],
                                    ph[:, :H],
                                    wsc[:, it : it + 1],
                                    None,
                                    mybir.AluOpType.mult,
                                )
                                nc.vector.tensor_add(
                                    hres[:, it, :], res[:, it, :], t1[:]
                                )
                    _sout(_t)

                    if not last_hop:
                        # z AllGather
                        _t = _sin(f"L{lay}.h{hop}.cc")
                        ccin = dramp.tile([OWN, H], BF, tag="ccin", bufs=2)
                        ccout = dramp.tile(
                            [N, H], BF, addr_space="Shared", tag="ccout", bufs=2
                        )
                        nc.sync.dma_start(
                            ccin[:].rearrange("(t p) f -> p t f", p=128), zbfo[:]
                        )
                        nc.gpsimd.collective_compute(
                            "AllGather",
                            mybir.AluOpType.bypass,
                            replica_groups=RG,
                            ins=[ccin.opt()],
                            outs=[ccout.opt()],
                        )
                        zn = zp.tile([128, C, H], BF, tag="z")
                        nc.sync.dma_start(
                            zn[:], ccout[:].rearrange("(c p) f -> p c f", p=128)
                        )
                        zprev = zn
                        _sout(_t)

                if lay == 0:
                    # ---- layer boundary: own k1^T/v1, AllGather them ----
                    _t = _sin("L0.kv")
                    hT1o = layp.tile([H, OWN], BF, tag="ht1o")
                    for it in range(OT):
                        ptr = pmp.tile([H, 128], F32, tag="pm", name=f"ptr{it}")
                        nc.tensor.transpose(ptr[:], hres[:, it, :], idf[:])
                        nc.vector.tensor_copy(hT1o[:, ts(it, 128)], ptr[:])
                    pq1 = pqp.tile([H, 512], F32, tag="pq", name="pq1")
                    nc.tensor.matmul(
                        pq1[:, :OWN], W2sb[0][:], hT1o[:], start=True, stop=True
                    )
                    q1To = layp.tile([H, OWN], BF, tag="qto")
                    nc.scalar.copy(q1To[:], pq1[:, :OWN])
                    pk1 = pqp.tile([H, 512], F32, tag="pq", name="pk1")
                    nc.tensor.matmul(
                        pk1[:, :OWN], W2sb[1][:], hT1o[:], start=True, stop=True
                    )
                    kto = layp.tile([H, OWN], BF, tag="kto")
                    nc.scalar.copy(kto[:], pk1[:, :OWN])
                    vob = ownp.tile([128, OT, H], BF, tag="vob")
                    avo1 = ownp.tile([128, OT, H], F32, tag="avo", name="avo1")
                    for it in range(OT):
                        pv = pmp.tile([128, H], F32, tag="pm", name=f"pv1_{it}")
                        nc.tensor.matmul(
                            pv[:], hT1o[:, ts(it, 128)], W2sb[2][:],
                            start=True, stop=True,
                        )
                        nc.vector.tensor_copy(vob[:, it, :], pv[:])
                        nc.scalar.mul(avo1[:, it, :], pv[:], ALPHA)
                    cckv_in = dramp.tile([2, KVW], BF, tag="cckvi")
                    nc.sync.dma_start(
                        cckv_in[0].rearrange("(p f) -> p f", p=H), kto[:]
                    )
                    nc.sync.dma_start(
                        cckv_in[1].rearrange("(t p f) -> p t f", p=128, f=H), vob[:]
                    )
                    cckv_out = dramp.tile(
                        [NC, 2, KVW], BF, addr_space="Shared", tag="cckvo"
                    )
                    nc.gpsimd.collective_compute(
                        "AllGather",
                        mybir.AluOpType.bypass,
                        replica_groups=RG,
                        ins=[cckv_in.opt()],
                        outs=[cckv_out.opt()],
                    )
                    kT8 = layp.tile([H, NC, OWN], BF, tag="kt8")
                    nc.sync.dma_start(
                        kT8[:], cckv_out[:, 0, :].rearrange("c (p f) -> p c f", p=H)
                    )
                    vsb1 = layp.tile([128, C, H + 1], BF, tag="v", name="vsb1")
                    for c in range(NC):
                        nc.sync.dma_start(
                            vsb1[:, c * OT : (c + 1) * OT, :H],
                            cckv_out[c, 1, :].rearrange(
                                "(t p f) -> p t f", p=128, f=H
                            ),
                        )
                    nc.vector.memset(vsb1[:, :, H], 1.0)
                    qTo, vsb, avo, res = q1To, vsb1, avo1, hres
                    _sout(_t)

            # ================= end stage =================
            _t = _sin("end.mm")
            graph = hfinal
            gTe = layp.tile([H + 1, OWN], F32, tag="gte")
            nc.vector.memset(gTe[H : H + 1, :], 1.0)
            for it in range(OT):
                pt = pmp.tile([H, 128], F32, tag="pm", name=f"gt{it}")
                nc.tensor.transpose(pt[:], graph[:, it, :], idf[:])
                nc.vector.tensor_copy(gTe[:H, ts(it, 128)], pt[:])

            sc = smp.tile([128, OT, G], F32, tag="sc", bufs=1)
            s2 = smp.tile([128, OT], F32, tag="s2", bufs=1)
            wsc2 = smp.tile([128, OT, G], F32, tag="wsc2", bufs=1)
            g2 = smp.tile([128, OT, H], F32, tag="g2", bufs=1)
            c2 = bigp.tile([128, OT, B], F32)
            for it in range(OT):
                pc = pmp.tile([128, G], F32, tag="pm", name=f"pc{it}")
                nc.tensor.matmul(
                    pc[:], gTe[:, ts(it, 128)], Wgsb[:], start=True, stop=True
                )
                nc.scalar.activation(
                    sc[:, it, :], pc[:], mybir.ActivationFunctionType.Exp
                )
                rs = smp.tile([128, 1], F32, tag="rs")
                nc.vector.tensor_reduce(
                    rs[:],
                    sc[:, it, :],
                    axis=mybir.AxisListType.X,
                    op=mybir.AluOpType.add,
                )
                nc.vector.reciprocal(rs[:], rs[:])
                nc.vector.tensor_scalar(
                    sc[:, it, :], sc[:, it, :], rs[:], None, mybir.AluOpType.mult
                )
                sq = smp.tile([128, G], F32, tag="sq")
                nc.vector.tensor_mul(sq[:], sc[:, it, :], sc[:, it, :])
                nc.vector.tensor_reduce(
                    s2[:, it : it + 1],
                    sq[:],
                    axis=mybir.AxisListType.X,
                    op=mybir.AluOpType.add,
                )
                nc.vector.tensor_scalar(
                    wsc2[:, it, :],
                    sc[:, it, :],
                    wpsb[:, it : it + 1],
                    None,
                    mybir.AluOpType.mult,
                )
                nc.vector.tensor_mul(g2[:, it, :], graph[:, it, :], graph[:, it, :])
                nc.vector.tensor_scalar(
                    c2[:, it, :],
                    cnt[:, it, :],
                    s2[:, it : it + 1],
                    None,
                    mybir.AluOpType.mult,
                )

            packsb = smp.tile([128, 5, H], F32, tag="pack", bufs=1)
            pS = pmp.tile([G, H], F32, tag="pm", name="pS")
            for it in range(OT):
                nc.tensor.matmul(
                    pS[:],
                    wsc2[:, it, :],
                    graph[:, it, :],
                    start=(it == 0),
                    stop=(it == OT - 1),
                )
            nc.vector.memset(packsb[:, 4, :], 0.0)
            nc.vector.tensor_copy(packsb[:G, 4, :], pS[:])
            for bt in range(2):
                pmean = pmp.tile([128, H], F32, tag="pm", name=f"pmean{bt}")
                for it in range(OT):
                    nc.tensor.matmul(
                        pmean[:],
                        cnt[:, it, ts(bt, 128)],
                        graph[:, it, :],
                        start=(it == 0),
                        stop=(it == OT - 1),
                    )
                nc.scalar.copy(packsb[:, bt, :], pmean[:])
            for bt in range(2):
                pE2 = pmp.tile([128, H], F32, tag="pm", name=f"pE2{bt}")
                for it in range(OT):
                    nc.tensor.matmul(
                        pE2[:],
                        c2[:, it, ts(bt, 128)],
                        g2[:, it, :],
                        start=(it == 0),
                        stop=(it == OT - 1),
                    )
                nc.scalar.copy(packsb[:, 2 + bt, :], pE2[:])
            _sout(_t)

            _t = _sin("end.ar")
            arin = dramp.tile([2 * B + G, H], F32)
            arout = dramp.tile([2 * B + G, H], F32, addr_space="Shared")
            nc.sync.dma_start(
                arin[: 2 * B, :].rearrange("(x p) f -> p x f", p=128),
                packsb[:, :4, :],
            )
            nc.sync.dma_start(arin[2 * B :, :], packsb[:G, 4, :])
            nc.gpsimd.collective_compute(
                "AllReduce",
                mybir.AluOpType.add,
                replica_groups=RG,
                ins=[arin.opt()],
                outs=[arout.opt()],
            )
            _sout(_t)

            _t = _sin("end.fin")
            # m_e2: [mean_bt0, mean_bt1, E2_bt0, E2_bt1] in one DMA
            m_e2 = smp.tile([128, 4, H], F32, tag="me2", bufs=1)
            nc.sync.dma_start(
                m_e2[:], arout[: 2 * B, :].rearrange("(x p) f -> p x f", p=128)
            )
            Ssb = smp.tile([128, G, H], F32, tag="Ssb", bufs=1)
            for g in range(G):
                nc.sync.dma_start(
                    Ssb[:, g, :],
                    arout[2 * B + g : 2 * B + g + 1, :].partition_broadcast(128),
                )
            mm = m_e2[:, 0:2, :]
            ee = m_e2[:, 2:4, :]
            var = smp.tile([128, 2, H], F32, tag="var", bufs=1)
            nc.vector.tensor_mul(var[:], mm, mm)
            nc.vector.tensor_sub(var[:], ee, var[:])
            sd = smp.tile([128, 2, H], F32, tag="sd", bufs=1)
            nc.scalar.activation(
                sd[:].rearrange("p a b -> p (a b)"),
                var[:].rearrange("p a b -> p (a b)"),
                mybir.ActivationFunctionType.Sqrt,
                bias=epsc[:],
            )
            Am = smp.tile([128, 2, H], F32, tag="Am", bufs=1)
            nc.vector.reciprocal(Am[:], sd[:])
            for bt in range(2):
                nc.vector.tensor_mul(Am[:, bt, :], Am[:, bt, :], gam[:])
            Bc = smp.tile([128, 2, H], F32, tag="Bc", bufs=1)
            nc.vector.tensor_mul(Bc[:], Am[:], mm)
            nc.vector.tensor_scalar_mul(Bc[:], Bc[:], -W_scalar)
            for bt in range(2):
                nc.vector.tensor_add(Bc[:, bt, :], Bc[:, bt, :], bet[:])
            for bt in range(2):
                og = smp.tile([128, G, H], F32, tag="og")
                nc.vector.tensor_mul(
                    og[:], Ssb[:], Am[:, bt, None, :].broadcast_to([128, G, H])
                )
                nc.vector.tensor_add(
                    og[:], og[:], Bc[:, bt, None, :].broadcast_to([128, G, H])
                )
                nc.sync.dma_start(o_out.ap()[ts(bt, 128), :, :], og[:])
            _sout(_t)

    nc.compile()
    return nc


def _prep_inputs(inputs):
    cate = np.asarray(inputs["cate_list"])
    adj = np.asarray(inputs["adj"], np.float32)
    emb = np.asarray(inputs["emb"], np.float32)
    Wq = np.asarray(inputs["Wq"], np.float32)
    Wk = np.asarray(inputs["Wk"], np.float32)
    Wv = np.asarray(inputs["Wv"], np.float32)
    Wg = np.asarray(inputs["Wg"], np.float32)
    bg = np.asarray(inputs["bg"], np.float32)
    Wp = np.asarray(inputs["Wp"], np.float32)
    bp = np.asarray(inputs["bp"], np.float32)
    gamma = np.asarray(inputs["gamma"], np.float32)
    beta = np.asarray(inputs["beta"], np.float32)

    adjP = np.zeros((N, N), np.float32)
    adjP[:NUM_CATES, :NUM_CATES] = adj
    idx = np.arange(NUM_CATES, N)
    adjP[idx, idx] = 1.0

    embP = np.zeros((N, H), np.float32)
    embP[:NUM_CATES] = emb

    # host precompute of layer-0 q/k/v (bf16 weights to match on-device numerics)
    Wq0 = Wq[0].astype(BF16).astype(np.float32)
    Wk0 = Wk[0].astype(BF16).astype(np.float32)
    Wv0 = Wv[0].astype(BF16).astype(np.float32)
    embB = embP.astype(BF16).astype(np.float32)
    k0T = np.ascontiguousarray((embB @ Wk0).T).astype(BF16)
    q0T = np.ascontiguousarray((embB @ Wq0).T).astype(BF16)
    v0 = embB @ Wv0
    v0_bf = v0.astype(BF16)

    WpP = np.zeros((N, 1), np.float32)
    WpP[:NUM_CATES] = Wp
    W = float(Wp.sum())
    beta_eff = (beta * W + bp).astype(np.float32).reshape(1, H)
    Wg_ext = np.concatenate([Wg, bg.reshape(1, G)], axis=0).astype(np.float32)

    counts = np.zeros((B, N), np.float32)
    bi = np.repeat(np.arange(B), L)
    ci = cate.reshape(-1).astype(np.int64)
    msk = (ci != 0).astype(np.float32) / float(G * L)
    np.add.at(counts, (bi, ci), msk)
    countsT = np.ascontiguousarray(counts.T)

    in_maps = []
    for c in range(NC):
        sl = slice(c * OWN, (c + 1) * OWN)
        in_maps.append(
            {
                "k0T_bf": k0T,
                "q0To_bf": np.ascontiguousarray(q0T[:, sl]),
                "v0_bf": v0_bf,
                "avo0": np.ascontiguousarray(ALPHA * v0[sl]).astype(np.float32),
                "emb_own": np.ascontiguousarray(embP[sl]),
                "adjT_own": np.ascontiguousarray(adjP[sl].T).astype(BF16),
                "Wq2": Wq[1].astype(BF16),
                "Wk2": Wk[1].astype(BF16),
                "Wv2": Wv[1].astype(BF16),
                "Wg_ext": Wg_ext,
                "Wp_own": np.ascontiguousarray(WpP[sl]),
                "countsT_own": np.ascontiguousarray(countsT[sl]),
                "gamma_row": gamma.reshape(1, H).astype(np.float32),
                "beta_eff_row": beta_eff,
            }
        )
    return in_maps, W


_NC_CACHE = {}


def kernel(**inputs) -> np.ndarray:
    in_maps, W = _prep_inputs(inputs)
    key = round(W, 10)
    nc = _NC_CACHE.get(key)
    if nc is None:
        nc = _build_nc(W)
        _NC_CACHE[key] = nc
    res = run_bass_kernel_spmd(nc, in_maps, core_ids=list(range(NC)))
    return np.asarray(res.results[0]["out"], np.float32)
